# revision 17
# baseline (speedup 1.0000x reference)
"""Trainium2 Bass kernel for DNNLSTMWithAttention.

Math (reference collapsed):
  d1 = relu(x @ fc1_w.T + fc1_b)
  d2 = relu(d1 @ fc2_w.T + fc2_b)
  d3 = d2 @ fc3_w.T + fc3_b
  gates = [x, d3] @ W_ih.T + b_ih + b_hh     (PyTorch order i,f,g,o; f unused)
  c = sigmoid(i) * tanh(g)
  h = sigmoid(o) * tanh(c)
  out = h @ fc_w.T + fc_b                    (attention over seq_len=1 == identity)

Structural simplifications (exact for ALL inputs):
  - softmax over a single score == 1 -> ctx == h; attn_w/attn_b never matter
  - f gate multiplies c0 == 0 -> never computed
  - W_hh @ h0 == 0 -> W_hh never matters
When all biases are zero (true for this problem's setup_inputs), fc3 folds
into the gate weights: gates_d = (W_ihd @ fc3_w) @ d2.

Layout: pure data parallel over 8 cores (16384 rows each).  Per core,
512-row macro tiles.  x is DMA-loaded with fp32->fp16 cast (SWDGE), then
transposed on-chip by TensorE (128x128 blocks vs fp16 identity) so features
land on partitions.  All matmuls run in fp16 with fp32 PSUM accumulation;
weights are tiny, pre-transposed on host, and resident in SBUF.
"""

import numpy as np

IN, OUT, H = 512, 128, 128
B = 131072
NCORES = 8
ROWS_PER_CORE = B // NCORES  # 16384
MACRO = 512                  # batch rows per macro tile (4 subtiles of 128)

_nc_cache: dict = {}


def _ensure_path():
    import sys
    try:
        import concourse.bass  # noqa: F401
    except ImportError:
        sys.path.insert(0, "/opt/trn_rl_repo")


def _build(n_rows: int, tmode: str = "pe"):
    """Build the SPMD Bass program for one core processing n_rows rows.

    tmode='pe':  transpose x via TensorE fp32 packed-pair blocks
    tmode='dma': transpose x via DMA xbar (16 fp16 128x128 blocks/macro)
    """
    _ensure_path()
    from contextlib import ExitStack

    import concourse.bass as bass
    import concourse.tile as tile
    from concourse import bacc, mybir

    fp16 = mybir.dt.float16
    fp32 = mybir.dt.float32
    AF = mybir.ActivationFunctionType
    ts = bass.ts

    assert n_rows % MACRO == 0
    n_macro = n_rows // MACRO

    nc = bacc.Bacc(
        "TRN2", target_bir_lowering=False, debug=False, num_devices=NCORES
    )

    x = nc.dram_tensor("x", [n_rows, IN], fp32, kind="ExternalInput").ap()
    # wbig: [128, 4*512]; col k*512+m, row p  <->  W_bigT[k*128+p, m]
    # where W_big rows m = [fc1_w | W_ih_i | W_ih_g | W_ih_o][:, :512]
    wbig = nc.dram_tensor("wbig", [128, 2048], fp16, kind="ExternalInput").ap()
    fc2t = nc.dram_tensor("fc2t", [128, 64], fp16, kind="ExternalInput").ap()
    # (W_ihd[i,g,o] @ fc3_w).T : [64, 384]
    wgdt = nc.dram_tensor("wgdt", [64, 384], fp16, kind="ExternalInput").ap()
    fct = nc.dram_tensor("fct", [H, OUT], fp16, kind="ExternalInput").ap()
    ident = nc.dram_tensor("ident", [128, 128], fp32, kind="ExternalInput").ap()
    out = nc.dram_tensor("out", [n_rows, OUT], fp32, kind="ExternalOutput").ap()

    # per-macro views: partition = row within 128-subtile, free = (subtile j, col)
    xin = x.rearrange("(m j p) f -> m p j f", j=4, p=128)
    outr = out.rearrange("(m j p) o -> m p j o", j=4, p=128)

    with tile.TileContext(nc) as tc, ExitStack() as ctx:
        cpool = ctx.enter_context(tc.tile_pool(name="consts", bufs=1))
        sb = ctx.enter_context(tc.tile_pool(name="sb", bufs=3))
        ps_xt = ctx.enter_context(tc.tile_pool(name="ps_xt", bufs=1, space="PSUM"))
        ps_g = ctx.enter_context(tc.tile_pool(name="ps_g", bufs=4, space="PSUM"))
        ps_mm = ctx.enter_context(tc.tile_pool(name="ps_mm", bufs=3, space="PSUM"))

        wbig_sb = cpool.tile([128, 2048], fp16, tag="wbig")
        nc.sync.dma_start(wbig_sb[:], wbig)
        fc2t_sb = cpool.tile([128, 64], fp16, tag="fc2t")
        nc.sync.dma_start(fc2t_sb[:], fc2t)
        wgdt_sb = cpool.tile([64, 384], fp16, tag="wgdt")
        nc.sync.dma_start(wgdt_sb[:], wgdt)
        fct_sb = cpool.tile([H, OUT], fp16, tag="fct")
        nc.sync.dma_start(fct_sb[:], fct)
        ident_sb = cpool.tile([128, 128], fp32, tag="ident")
        nc.sync.dma_start(ident_sb[:], ident)

        # Software-pipelined emission.  Engine instruction streams execute
        # in order, so emission order is the schedule: each iteration emits
        # macro m's dependency-light head (transposes, d1, gates_x) first,
        # finishes macro m-1 in the gaps, then macro m's dependent tail.
        def emit_head(m):
            x16 = sb.tile([128, 2048], fp16, tag="x16")
            nc.gpsimd.dma_start(out=x16[:], in_=xin[m])
            return x16

        x16_cur = emit_head(0)
        x16_next = emit_head(1) if n_macro > 1 else None
        prev = None  # state of macro m-1: dict(pg=[...], m=...)

        for m in range(n_macro):
            x16 = x16_cur
            if tmode == "pe":
                # ---- transposes: two features packed per fp32 word; the
                # feature permutation is folded into the host weight layout
                x32 = x16[:].bitcast(fp32)  # [128, 1024]
                xt = sb.tile([128, 1024], fp32, tag="xt")
                for wb in range(2):
                    pxt = ps_xt.tile([128, 512], fp32, tag="pxt")
                    for j in range(4):
                        nc.tensor.transpose(
                            pxt[:, ts(j, 128)],
                            x32[:, j * 256 + wb * 128 : j * 256 + (wb + 1) * 128],
                            ident_sb[:],
                        )
                    nc.vector.tensor_copy(xt[:, ts(wb, 512)], pxt[:])
                # fp16 col = wb*1024 + j*256 + 2*b + h ; K-chunk kk=(wb,h)
                xtr = xt[:].bitcast(fp16).rearrange(
                    "p (wb j b two) -> p wb j b two", wb=2, j=4, two=2
                )
                rhs = [xtr[:, kk // 2, :, :, kk % 2] for kk in range(4)]
            else:
                # ---- transposes on the DMA xbar: frees TensorE entirely
                xt = sb.tile([128, 2048], fp16, tag="xt")
                for k in range(4):
                    for j in range(4):
                        nc.sync.dma_start(
                            out=xt[:, k * 512 + j * 128 : k * 512 + (j + 1) * 128],
                            in_=x16[:, j * 512 + k * 128 : j * 512 + (k + 1) * 128],
                            transpose=True,
                        )
                rhs = [xt[:, ts(k, 512)] for k in range(4)]

            # ---- d1 psum: [128 feat, 512 batch] ----
            pd1 = ps_mm.tile([128, 512], fp32, tag="pmm")
            for k in range(4):
                nc.tensor.matmul(
                    pd1[:],
                    wbig_sb[:, k * 512 : k * 512 + 128],
                    rhs[k],
                    start=(k == 0),
                    stop=(k == 3),
                )

            # ---- previous macro: evict gates (ACT) while PE runs gates_x
            if prev is not None:
                si = sb.tile([128, 512], fp16, tag="si")
                nc.scalar.activation(si[:], prev["pg"][0][:], AF.Sigmoid)
                tg = sb.tile([128, 512], fp16, tag="tg")
                nc.scalar.activation(tg[:], prev["pg"][1][:], AF.Tanh)
                so = sb.tile([128, 512], fp16, tag="so")
                nc.scalar.activation(so[:], prev["pg"][2][:], AF.Sigmoid)
                prev.update(si=si, tg=tg, so=so)

            # ---- gates_x (12 MMs, dep only on xt) ----
            pg = []
            for g in range(3):
                p = ps_g.tile([128, 512], fp32, tag="pg")
                for k in range(4):
                    nc.tensor.matmul(
                        p[:],
                        wbig_sb[:, k * 512 + 128 + g * 128 : k * 512 + 256 + g * 128],
                        rhs[k],
                        start=(k == 0),
                        stop=False,
                        skip_group_check=True,
                    )
                pg.append(p)

            # ---- this macro: d1 relu on ACT; previous macro: lstm cell math
            d1 = sb.tile([128, 512], fp16, tag="d1")
            nc.scalar.activation(d1[:], pd1[:], AF.Relu)
            if prev is not None:
                c = sb.tile([128, 512], fp16, tag="c")
                nc.vector.tensor_mul(c[:], prev["si"][:], prev["tg"][:])
                tc_ = sb.tile([128, 512], fp16, tag="tc")
                nc.scalar.activation(tc_[:], c[:], AF.Tanh)
                h = sb.tile([128, 512], fp16, tag="h")
                nc.vector.tensor_mul(h[:], prev["so"][:], tc_[:])
                prev["h"] = h

            # ---- fc2 ----
            pd2 = ps_mm.tile([64, 512], fp32, tag="pmm")
            nc.tensor.matmul(pd2[:], fc2t_sb[:], d1[:], start=True, stop=True)

            # ---- previous macro: final fc + store ----
            if prev is not None:
                pout = ps_mm.tile([128, 512], fp32, tag="pmm")
                for j in range(4):
                    nc.tensor.matmul(
                        pout[:, ts(j, 128)],
                        prev["h"][:, ts(j, 128)],
                        fct_sb[:],
                        start=True,
                        stop=True,
                    )
                osb = sb.tile([128, 512], fp32, tag="osb")
                nc.vector.tensor_copy(osb[:], pout[:])
                nc.sync.dma_start(out=outr[prev["m"]], in_=osb[:])

            # ---- d2 relu on DVE, then gate d-contributions close groups ----
            d2 = sb.tile([64, 512], fp16, tag="d2")
            nc.vector.tensor_scalar_max(d2[:], pd2[:], 0.0)
            for g in range(3):
                nc.tensor.matmul(
                    pg[g][:],
                    wgdt_sb[:, ts(g, 128)],
                    d2[:],
                    start=False,
                    stop=True,
                    skip_group_check=True,
                )

            prev = {"pg": pg, "m": m}
            x16_cur = x16_next
            x16_next = emit_head(m + 2) if m + 2 < n_macro else None

        # ---- epilogue: finish the last macro ----
        si = sb.tile([128, 512], fp16, tag="si")
        nc.scalar.activation(si[:], prev["pg"][0][:], AF.Sigmoid)
        tg = sb.tile([128, 512], fp16, tag="tg")
        nc.scalar.activation(tg[:], prev["pg"][1][:], AF.Tanh)
        so = sb.tile([128, 512], fp16, tag="so")
        nc.scalar.activation(so[:], prev["pg"][2][:], AF.Sigmoid)
        c = sb.tile([128, 512], fp16, tag="c")
        nc.vector.tensor_mul(c[:], si[:], tg[:])
        tc_ = sb.tile([128, 512], fp16, tag="tc")
        nc.scalar.activation(tc_[:], c[:], AF.Tanh)
        h = sb.tile([128, 512], fp16, tag="h")
        nc.vector.tensor_mul(h[:], so[:], tc_[:])
        pout = ps_mm.tile([128, 512], fp32, tag="pmm")
        for j in range(4):
            nc.tensor.matmul(
                pout[:, ts(j, 128)], h[:, ts(j, 128)], fct_sb[:],
                start=True, stop=True,
            )
        osb = sb.tile([128, 512], fp32, tag="osb")
        nc.vector.tensor_copy(osb[:], pout[:])
        nc.sync.dma_start(out=outr[prev["m"]], in_=osb[:])

    nc.compile()
    return nc


def _prep_consts(fc1_w, fc2_w, fc3_w, W_ih, fc_w, tmode="pe"):
    wi = W_ih[0:128, :IN]
    wg = W_ih[256:384, :IN]
    wo = W_ih[384:512, :IN]
    wbigT = np.concatenate([fc1_w, wi, wg, wo], axis=0).T.astype(np.float32)
    # feature permutation matching the packed-pair transpose: K-chunk
    # kk=(wb,h) row p holds feature 2*(wb*128+p)+h
    if tmode == "pe":
        perm = np.empty(IN, np.int64)
        for kk in range(4):
            wb, h = kk // 2, kk % 2
            p = np.arange(128)
            perm[kk * 128 + p] = 2 * (wb * 128 + p) + h
        wbigT = wbigT[perm]
    # [512, 512] -> [128, 2048] with col k*512+m, row p <-> wbigT[k*128+p, m]
    wbig = np.ascontiguousarray(
        wbigT.reshape(4, 128, 512).transpose(1, 0, 2).reshape(128, 2048)
    ).astype(np.float16)

    wd = np.concatenate(
        [W_ih[0:128, IN:], W_ih[256:384, IN:], W_ih[384:512, IN:]], axis=0
    )  # [384, 128]
    wgdt = np.ascontiguousarray((wd @ fc3_w).T).astype(np.float16)  # [64, 384]
    fc2t = np.ascontiguousarray(fc2_w.T).astype(np.float16)
    fct = np.ascontiguousarray(fc_w.T).astype(np.float16)
    return wbig, fc2t, wgdt, fct


def _numpy_fallback(x, fc1_w, fc1_b, fc2_w, fc2_b, fc3_w, fc3_b,
                    W_ih, b_ih, b_hh, fc_w, fc_b):
    def sig(v):
        return 1.0 / (1.0 + np.exp(-v))

    out = np.empty((x.shape[0], OUT), np.float32)
    bs = 8192
    for s in range(0, x.shape[0], bs):
        xe = x[s : s + bs]
        d = np.maximum(xe @ fc1_w.T + fc1_b, 0.0)
        d = np.maximum(d @ fc2_w.T + fc2_b, 0.0)
        d = d @ fc3_w.T + fc3_b
        xc = np.concatenate([xe, d], axis=1)
        gates = xc @ W_ih.T + b_ih + b_hh
        i, f, g, o = np.split(gates, 4, axis=1)
        c = sig(i) * np.tanh(g)
        h = sig(o) * np.tanh(c)
        out[s : s + bs] = h @ fc_w.T + fc_b
    return out


def kernel(x, fc1_w, fc1_b, fc2_w, fc2_b, fc3_w, fc3_b,
           W_ih, W_hh, b_ih, b_hh, attn_w, attn_b, fc_w, fc_b):
    x = np.asarray(x, np.float32)
    arrs = dict(
        fc1_w=np.asarray(fc1_w, np.float32), fc1_b=np.asarray(fc1_b, np.float32),
        fc2_w=np.asarray(fc2_w, np.float32), fc2_b=np.asarray(fc2_b, np.float32),
        fc3_w=np.asarray(fc3_w, np.float32), fc3_b=np.asarray(fc3_b, np.float32),
        W_ih=np.asarray(W_ih, np.float32), b_ih=np.asarray(b_ih, np.float32),
        b_hh=np.asarray(b_hh, np.float32), fc_w=np.asarray(fc_w, np.float32),
        fc_b=np.asarray(fc_b, np.float32),
    )
    # attn_w/attn_b/W_hh provably never affect the output (softmax over one
    # score == 1; h0 == c0 == 0).  Nonzero biases take the exact host path.
    if any(
        np.any(arrs[k]) for k in ("fc1_b", "fc2_b", "fc3_b", "b_ih", "b_hh", "fc_b")
    ):
        return _numpy_fallback(x, **{k: arrs[k] for k in arrs})

    tmode = "pe"
    wbig, fc2t, wgdt, fct = _prep_consts(
        arrs["fc1_w"], arrs["fc2_w"], arrs["fc3_w"], arrs["W_ih"], arrs["fc_w"],
        tmode,
    )
    ident = np.eye(128, dtype=np.float32)

    n_rows = x.shape[0] // NCORES
    if (n_rows, tmode) not in _nc_cache:
        _nc_cache[(n_rows, tmode)] = _build(n_rows, tmode)
    nc = _nc_cache[(n_rows, tmode)]

    _ensure_path()
    from concourse.bass_utils import run_bass_kernel_spmd

    xs = np.ascontiguousarray(x.reshape(NCORES, n_rows, IN))
    in_maps = [
        {"x": xs[i], "wbig": wbig, "fc2t": fc2t, "wgdt": wgdt, "fct": fct,
         "ident": ident}
        for i in range(NCORES)
    ]
    res = run_bass_kernel_spmd(nc, in_maps, core_ids=list(range(NCORES)))
    return np.concatenate([res.results[i]["out"] for i in range(NCORES)], axis=0)


# revision 19
# speedup vs baseline: 4.2444x; 4.2444x over previous
"""Trainium2 Bass kernel for DNNLSTMWithAttention.

Math (reference collapsed):
  d1 = relu(x @ fc1_w.T + fc1_b)
  d2 = relu(d1 @ fc2_w.T + fc2_b)
  d3 = d2 @ fc3_w.T + fc3_b
  gates = [x, d3] @ W_ih.T + b_ih + b_hh     (PyTorch order i,f,g,o; f unused)
  c = sigmoid(i) * tanh(g)
  h = sigmoid(o) * tanh(c)
  out = h @ fc_w.T + fc_b                    (attention over seq_len=1 == identity)

Structural simplifications (exact for ALL inputs):
  - softmax over a single score == 1 -> ctx == h; attn_w/attn_b never matter
  - f gate multiplies c0 == 0 -> never computed
  - W_hh @ h0 == 0 -> W_hh never matters
When all biases are zero (true for this problem's setup_inputs), fc3 folds
into the gate weights: gates_d = (W_ihd @ fc3_w) @ d2.

Layout: pure data parallel over 8 cores (16384 rows each).  Per core,
512-row macro tiles.  x is DMA-loaded with fp32->fp16 cast (SWDGE), then
transposed on-chip by TensorE (128x128 blocks vs fp16 identity) so features
land on partitions.  All matmuls run in fp16 with fp32 PSUM accumulation;
weights are tiny, pre-transposed on host, and resident in SBUF.
"""

import numpy as np

IN, OUT, H = 512, 128, 128
B = 131072
NCORES = 8
ROWS_PER_CORE = B // NCORES  # 16384
MACRO = 512                  # batch rows per macro tile (4 subtiles of 128)

_nc_cache: dict = {}


def _ensure_path():
    import sys
    try:
        import concourse.bass  # noqa: F401
    except ImportError:
        sys.path.insert(0, "/opt/trn_rl_repo")


def _build(n_rows: int, tmode: str = "pe"):
    """Build the SPMD Bass program for one core processing n_rows rows.

    tmode='pe':  transpose x via TensorE fp32 packed-pair blocks
    tmode='dma': transpose x via DMA xbar (16 fp16 128x128 blocks/macro)
    """
    _ensure_path()
    from contextlib import ExitStack

    import concourse.bass as bass
    import concourse.tile as tile
    from concourse import bacc, mybir

    fp16 = mybir.dt.float16
    fp32 = mybir.dt.float32
    AF = mybir.ActivationFunctionType
    ts = bass.ts

    assert n_rows % MACRO == 0
    n_macro = n_rows // MACRO

    nc = bacc.Bacc(
        "TRN2", target_bir_lowering=False, debug=False, num_devices=NCORES
    )

    x = nc.dram_tensor("x", [n_rows, IN], fp32, kind="ExternalInput").ap()
    # wbig: [128, 4*512]; col k*512+m, row p  <->  W_bigT[k*128+p, m]
    # where W_big rows m = [fc1_w | W_ih_i | W_ih_g | W_ih_o][:, :512]
    wbig = nc.dram_tensor("wbig", [128, 2048], fp16, kind="ExternalInput").ap()
    fc2t = nc.dram_tensor("fc2t", [128, 64], fp16, kind="ExternalInput").ap()
    # (W_ihd[i,g,o] @ fc3_w).T : [64, 384]
    wgdt = nc.dram_tensor("wgdt", [64, 384], fp16, kind="ExternalInput").ap()
    fct = nc.dram_tensor("fct", [H, OUT], fp16, kind="ExternalInput").ap()
    ident = nc.dram_tensor("ident", [128, 128], fp32, kind="ExternalInput").ap()
    out = nc.dram_tensor("out", [n_rows, OUT], fp32, kind="ExternalOutput").ap()

    # per-macro views: partition = row within 128-subtile, free = (subtile j, col)
    xin = x.rearrange("(m j p) f -> m p j f", j=4, p=128)
    outr = out.rearrange("(m j p) o -> m p j o", j=4, p=128)

    with tile.TileContext(nc) as tc, ExitStack() as ctx:
        cpool = ctx.enter_context(tc.tile_pool(name="consts", bufs=1))
        sb = ctx.enter_context(tc.tile_pool(name="sb", bufs=3))
        ps_xt = ctx.enter_context(tc.tile_pool(name="ps_xt", bufs=2, space="PSUM"))
        ps_g = ctx.enter_context(tc.tile_pool(name="ps_g", bufs=3, space="PSUM"))
        ps_mm = ctx.enter_context(tc.tile_pool(name="ps_mm", bufs=3, space="PSUM"))

        wbig_sb = cpool.tile([128, 2048], fp16, tag="wbig")
        nc.sync.dma_start(wbig_sb[:], wbig)
        fc2t_sb = cpool.tile([128, 64], fp16, tag="fc2t")
        nc.sync.dma_start(fc2t_sb[:], fc2t)
        wgdt_sb = cpool.tile([64, 384], fp16, tag="wgdt")
        nc.sync.dma_start(wgdt_sb[:], wgdt)
        fct_sb = cpool.tile([H, OUT], fp16, tag="fct")
        nc.sync.dma_start(fct_sb[:], fct)
        ident_sb = cpool.tile([128, 128], fp32, tag="ident")
        nc.sync.dma_start(ident_sb[:], ident)

        # Software-pipelined emission.  Engine instruction streams execute
        # in order, so emission order is the schedule: each iteration emits
        # macro m's dependency-light head (transposes, d1, gates_x) first,
        # finishes macro m-1 in the gaps, then macro m's dependent tail.
        def emit_head(m):
            # four per-subtile DMAs: transposes of subtile j only wait on
            # load j, so the pipeline starts (and recovers) faster
            x16 = sb.tile([128, 2048], fp16, tag="x16")
            for j in range(4):
                nc.gpsimd.dma_start(
                    out=x16[:, ts(j, 512)], in_=xin[m, :, j]
                )
            return x16

        x16_cur = emit_head(0)
        x16_next = emit_head(1) if n_macro > 1 else None
        prev = None  # state of macro m-1: dict(pg=[...], m=...)

        for m in range(n_macro):
            x16 = x16_cur
            if tmode == "pe":
                # ---- transposes: two features packed per fp32 word; the
                # feature permutation is folded into the host weight layout
                x32 = x16[:].bitcast(fp32)  # [128, 1024]
                xt = sb.tile([128, 1024], fp32, tag="xt")
                for wb in range(2):
                    pxt = ps_xt.tile([128, 512], fp32, tag="pxt")
                    for j in range(4):
                        nc.tensor.transpose(
                            pxt[:, ts(j, 128)],
                            x32[:, j * 256 + wb * 128 : j * 256 + (wb + 1) * 128],
                            ident_sb[:],
                        )
                    nc.vector.tensor_copy(xt[:, ts(wb, 512)], pxt[:])
                # fp16 col = wb*1024 + j*256 + 2*b + h ; K-chunk kk=(wb,h)
                xtr = xt[:].bitcast(fp16).rearrange(
                    "p (wb j b two) -> p wb j b two", wb=2, j=4, two=2
                )
                rhs = [xtr[:, kk // 2, :, :, kk % 2] for kk in range(4)]
            else:
                # ---- transposes on the DMA xbar: frees TensorE entirely
                xt = sb.tile([128, 2048], fp16, tag="xt")
                for k in range(4):
                    for j in range(4):
                        nc.sync.dma_start(
                            out=xt[:, k * 512 + j * 128 : k * 512 + (j + 1) * 128],
                            in_=x16[:, j * 512 + k * 128 : j * 512 + (k + 1) * 128],
                            transpose=True,
                        )
                rhs = [xt[:, ts(k, 512)] for k in range(4)]

            # ---- d1 psum: [128 feat, 512 batch] ----
            pd1 = ps_mm.tile([128, 512], fp32, tag="pmm")
            for k in range(4):
                nc.tensor.matmul(
                    pd1[:],
                    wbig_sb[:, k * 512 : k * 512 + 128],
                    rhs[k],
                    start=(k == 0),
                    stop=(k == 3),
                )

            # ---- previous macro: evict gates (ACT) while PE runs gates_x
            if prev is not None:
                si = sb.tile([128, 512], fp16, tag="si")
                nc.scalar.activation(si[:], prev["pg"][0][:], AF.Sigmoid)
                tg = sb.tile([128, 512], fp16, tag="tg")
                nc.scalar.activation(tg[:], prev["pg"][1][:], AF.Tanh)
                so = sb.tile([128, 512], fp16, tag="so")
                nc.scalar.activation(so[:], prev["pg"][2][:], AF.Sigmoid)
                prev.update(si=si, tg=tg, so=so)

            # ---- gates_x (12 MMs, dep only on xt) ----
            pg = []
            for g in range(3):
                p = ps_g.tile([128, 512], fp32, tag="pg")
                for k in range(4):
                    nc.tensor.matmul(
                        p[:],
                        wbig_sb[:, k * 512 + 128 + g * 128 : k * 512 + 256 + g * 128],
                        rhs[k],
                        start=(k == 0),
                        stop=False,
                        skip_group_check=True,
                    )
                pg.append(p)

            # ---- this macro: d1 relu on ACT; previous macro: lstm cell math
            d1 = sb.tile([128, 512], fp16, tag="d1")
            nc.scalar.activation(d1[:], pd1[:], AF.Relu)
            if prev is not None:
                c = sb.tile([128, 512], fp16, tag="c")
                nc.vector.tensor_mul(c[:], prev["si"][:], prev["tg"][:])
                tc_ = sb.tile([128, 512], fp16, tag="tc")
                nc.scalar.activation(tc_[:], c[:], AF.Tanh)
                h = sb.tile([128, 512], fp16, tag="h")
                nc.vector.tensor_mul(h[:], prev["so"][:], tc_[:])
                prev["h"] = h

            # ---- fc2 ----
            pd2 = ps_mm.tile([64, 512], fp32, tag="pmm")
            nc.tensor.matmul(pd2[:], fc2t_sb[:], d1[:], start=True, stop=True)

            # ---- previous macro: final fc + store ----
            if prev is not None:
                pout = ps_mm.tile([128, 512], fp32, tag="pmm")
                for j in range(4):
                    nc.tensor.matmul(
                        pout[:, ts(j, 128)],
                        prev["h"][:, ts(j, 128)],
                        fct_sb[:],
                        start=True,
                        stop=True,
                    )
                osb = sb.tile([128, 512], fp32, tag="osb")
                nc.vector.tensor_copy(osb[:], pout[:])
                nc.sync.dma_start(out=outr[prev["m"]], in_=osb[:])

            # ---- d2 relu on DVE, then gate d-contributions close groups ----
            d2 = sb.tile([64, 512], fp16, tag="d2")
            nc.vector.tensor_scalar_max(d2[:], pd2[:], 0.0)
            for g in range(3):
                nc.tensor.matmul(
                    pg[g][:],
                    wgdt_sb[:, ts(g, 128)],
                    d2[:],
                    start=False,
                    stop=True,
                    skip_group_check=True,
                )

            prev = {"pg": pg, "m": m}
            x16_cur = x16_next
            x16_next = emit_head(m + 2) if m + 2 < n_macro else None

        # ---- epilogue: finish the last macro ----
        si = sb.tile([128, 512], fp16, tag="si")
        nc.scalar.activation(si[:], prev["pg"][0][:], AF.Sigmoid)
        tg = sb.tile([128, 512], fp16, tag="tg")
        nc.scalar.activation(tg[:], prev["pg"][1][:], AF.Tanh)
        so = sb.tile([128, 512], fp16, tag="so")
        nc.scalar.activation(so[:], prev["pg"][2][:], AF.Sigmoid)
        c = sb.tile([128, 512], fp16, tag="c")
        nc.vector.tensor_mul(c[:], si[:], tg[:])
        tc_ = sb.tile([128, 512], fp16, tag="tc")
        nc.scalar.activation(tc_[:], c[:], AF.Tanh)
        h = sb.tile([128, 512], fp16, tag="h")
        nc.vector.tensor_mul(h[:], so[:], tc_[:])
        pout = ps_mm.tile([128, 512], fp32, tag="pmm")
        for j in range(4):
            nc.tensor.matmul(
                pout[:, ts(j, 128)], h[:, ts(j, 128)], fct_sb[:],
                start=True, stop=True,
            )
        osb = sb.tile([128, 512], fp32, tag="osb")
        nc.vector.tensor_copy(osb[:], pout[:])
        nc.sync.dma_start(out=outr[prev["m"]], in_=osb[:])

    nc.compile()
    return nc


def _prep_consts(fc1_w, fc2_w, fc3_w, W_ih, fc_w, tmode="pe"):
    wi = W_ih[0:128, :IN]
    wg = W_ih[256:384, :IN]
    wo = W_ih[384:512, :IN]
    wbigT = np.concatenate([fc1_w, wi, wg, wo], axis=0).T.astype(np.float32)
    # feature permutation matching the packed-pair transpose: K-chunk
    # kk=(wb,h) row p holds feature 2*(wb*128+p)+h
    if tmode == "pe":
        perm = np.empty(IN, np.int64)
        for kk in range(4):
            wb, h = kk // 2, kk % 2
            p = np.arange(128)
            perm[kk * 128 + p] = 2 * (wb * 128 + p) + h
        wbigT = wbigT[perm]
    # [512, 512] -> [128, 2048] with col k*512+m, row p <-> wbigT[k*128+p, m]
    wbig = np.ascontiguousarray(
        wbigT.reshape(4, 128, 512).transpose(1, 0, 2).reshape(128, 2048)
    ).astype(np.float16)

    wd = np.concatenate(
        [W_ih[0:128, IN:], W_ih[256:384, IN:], W_ih[384:512, IN:]], axis=0
    )  # [384, 128]
    wgdt = np.ascontiguousarray((wd @ fc3_w).T).astype(np.float16)  # [64, 384]
    fc2t = np.ascontiguousarray(fc2_w.T).astype(np.float16)
    fct = np.ascontiguousarray(fc_w.T).astype(np.float16)
    return wbig, fc2t, wgdt, fct


def _numpy_fallback(x, fc1_w, fc1_b, fc2_w, fc2_b, fc3_w, fc3_b,
                    W_ih, b_ih, b_hh, fc_w, fc_b):
    def sig(v):
        return 1.0 / (1.0 + np.exp(-v))

    out = np.empty((x.shape[0], OUT), np.float32)
    bs = 8192
    for s in range(0, x.shape[0], bs):
        xe = x[s : s + bs]
        d = np.maximum(xe @ fc1_w.T + fc1_b, 0.0)
        d = np.maximum(d @ fc2_w.T + fc2_b, 0.0)
        d = d @ fc3_w.T + fc3_b
        xc = np.concatenate([xe, d], axis=1)
        gates = xc @ W_ih.T + b_ih + b_hh
        i, f, g, o = np.split(gates, 4, axis=1)
        c = sig(i) * np.tanh(g)
        h = sig(o) * np.tanh(c)
        out[s : s + bs] = h @ fc_w.T + fc_b
    return out


def kernel(x, fc1_w, fc1_b, fc2_w, fc2_b, fc3_w, fc3_b,
           W_ih, W_hh, b_ih, b_hh, attn_w, attn_b, fc_w, fc_b):
    x = np.asarray(x, np.float32)
    arrs = dict(
        fc1_w=np.asarray(fc1_w, np.float32), fc1_b=np.asarray(fc1_b, np.float32),
        fc2_w=np.asarray(fc2_w, np.float32), fc2_b=np.asarray(fc2_b, np.float32),
        fc3_w=np.asarray(fc3_w, np.float32), fc3_b=np.asarray(fc3_b, np.float32),
        W_ih=np.asarray(W_ih, np.float32), b_ih=np.asarray(b_ih, np.float32),
        b_hh=np.asarray(b_hh, np.float32), fc_w=np.asarray(fc_w, np.float32),
        fc_b=np.asarray(fc_b, np.float32),
    )
    # attn_w/attn_b/W_hh provably never affect the output (softmax over one
    # score == 1; h0 == c0 == 0).  Nonzero biases take the exact host path.
    if any(
        np.any(arrs[k]) for k in ("fc1_b", "fc2_b", "fc3_b", "b_ih", "b_hh", "fc_b")
    ):
        return _numpy_fallback(x, **{k: arrs[k] for k in arrs})

    tmode = "pe"
    wbig, fc2t, wgdt, fct = _prep_consts(
        arrs["fc1_w"], arrs["fc2_w"], arrs["fc3_w"], arrs["W_ih"], arrs["fc_w"],
        tmode,
    )
    ident = np.eye(128, dtype=np.float32)

    n_rows = x.shape[0] // NCORES
    if (n_rows, tmode) not in _nc_cache:
        _nc_cache[(n_rows, tmode)] = _build(n_rows, tmode)
    nc = _nc_cache[(n_rows, tmode)]

    _ensure_path()
    from concourse.bass_utils import run_bass_kernel_spmd

    xs = np.ascontiguousarray(x.reshape(NCORES, n_rows, IN))
    in_maps = [
        {"x": xs[i], "wbig": wbig, "fc2t": fc2t, "wgdt": wgdt, "fct": fct,
         "ident": ident}
        for i in range(NCORES)
    ]
    res = run_bass_kernel_spmd(nc, in_maps, core_ids=list(range(NCORES)))
    return np.concatenate([res.results[i]["out"] for i in range(NCORES)], axis=0)


# revision 22
# speedup vs baseline: 4.2456x; 1.0003x over previous
"""Trainium2 Bass kernel for DNNLSTMWithAttention.

Math (reference collapsed):
  d1 = relu(x @ fc1_w.T + fc1_b)
  d2 = relu(d1 @ fc2_w.T + fc2_b)
  d3 = d2 @ fc3_w.T + fc3_b
  gates = [x, d3] @ W_ih.T + b_ih + b_hh     (PyTorch order i,f,g,o; f unused)
  c = sigmoid(i) * tanh(g)
  h = sigmoid(o) * tanh(c)
  out = h @ fc_w.T + fc_b                    (attention over seq_len=1 == identity)

Structural simplifications (exact for ALL inputs):
  - softmax over a single score == 1 -> ctx == h; attn_w/attn_b never matter
  - f gate multiplies c0 == 0 -> never computed
  - W_hh @ h0 == 0 -> W_hh never matters
When all biases are zero (true for this problem's setup_inputs), fc3 folds
into the gate weights: gates_d = (W_ihd @ fc3_w) @ d2.

Layout: pure data parallel over 8 cores (16384 rows each).  Per core,
512-row macro tiles.  x is DMA-loaded with fp32->fp16 cast (SWDGE), then
transposed on-chip by TensorE (128x128 blocks vs fp16 identity) so features
land on partitions.  All matmuls run in fp16 with fp32 PSUM accumulation;
weights are tiny, pre-transposed on host, and resident in SBUF.
"""

import numpy as np

IN, OUT, H = 512, 128, 128
B = 131072
NCORES = 8
ROWS_PER_CORE = B // NCORES  # 16384
MACRO = 512                  # batch rows per macro tile (4 subtiles of 128)

_nc_cache: dict = {}


def _ensure_path():
    import sys
    try:
        import concourse.bass  # noqa: F401
    except ImportError:
        sys.path.insert(0, "/opt/trn_rl_repo")


def _build(n_rows: int, tmode: str = "pe"):
    """Build the SPMD Bass program for one core processing n_rows rows.

    tmode='pe':  transpose x via TensorE fp32 packed-pair blocks
    tmode='dma': transpose x via DMA xbar (16 fp16 128x128 blocks/macro)
    """
    _ensure_path()
    from contextlib import ExitStack

    import concourse.bass as bass
    import concourse.tile as tile
    from concourse import bacc, mybir

    fp16 = mybir.dt.float16
    fp32 = mybir.dt.float32
    AF = mybir.ActivationFunctionType
    ts = bass.ts

    assert n_rows % MACRO == 0
    n_macro = n_rows // MACRO

    nc = bacc.Bacc(
        "TRN2", target_bir_lowering=False, debug=False, num_devices=NCORES
    )

    x = nc.dram_tensor("x", [n_rows, IN], fp32, kind="ExternalInput").ap()
    # wbig: [128, 4*512]; col k*512+m, row p  <->  W_bigT[k*128+p, m]
    # where W_big rows m = [fc1_w | W_ih_i | W_ih_g | W_ih_o][:, :512]
    wbig = nc.dram_tensor("wbig", [128, 2048], fp16, kind="ExternalInput").ap()
    fc2t = nc.dram_tensor("fc2t", [128, 64], fp16, kind="ExternalInput").ap()
    # (W_ihd[i,g,o] @ fc3_w).T : [64, 384]
    wgdt = nc.dram_tensor("wgdt", [64, 384], fp16, kind="ExternalInput").ap()
    fct = nc.dram_tensor("fct", [H, OUT], fp16, kind="ExternalInput").ap()
    ident = nc.dram_tensor("ident", [128, 128], fp32, kind="ExternalInput").ap()
    out = nc.dram_tensor("out", [n_rows, OUT], fp32, kind="ExternalOutput").ap()

    # per-macro views: partition = row within 128-subtile, free = (subtile j, col)
    xin = x.rearrange("(m j p) f -> m p j f", j=4, p=128)
    outr = out.rearrange("(m j p) o -> m p j o", j=4, p=128)

    with tile.TileContext(nc) as tc, ExitStack() as ctx:
        cpool = ctx.enter_context(tc.tile_pool(name="consts", bufs=1))
        sb = ctx.enter_context(tc.tile_pool(name="sb", bufs=3))
        ps_xt = ctx.enter_context(tc.tile_pool(name="ps_xt", bufs=2, space="PSUM"))
        ps_g = ctx.enter_context(tc.tile_pool(name="ps_g", bufs=3, space="PSUM"))
        ps_mm = ctx.enter_context(tc.tile_pool(name="ps_mm", bufs=3, space="PSUM"))

        # ident first: the very first PE transposes wait only on it + x(0,0)
        ident_sb = cpool.tile([128, 128], fp32, tag="ident")
        nc.sync.dma_start(ident_sb[:], ident)
        wbig_sb = cpool.tile([128, 2048], fp16, tag="wbig")
        nc.sync.dma_start(wbig_sb[:], wbig)
        fc2t_sb = cpool.tile([128, 64], fp16, tag="fc2t")
        nc.sync.dma_start(fc2t_sb[:], fc2t)
        wgdt_sb = cpool.tile([64, 384], fp16, tag="wgdt")
        nc.sync.dma_start(wgdt_sb[:], wgdt)
        fct_sb = cpool.tile([H, OUT], fp16, tag="fct")
        nc.sync.dma_start(fct_sb[:], fct)

        # Software-pipelined emission.  Engine instruction streams execute
        # in order, so emission order is the schedule: each iteration emits
        # macro m's dependency-light head (transposes, d1, gates_x) first,
        # finishes macro m-1 in the gaps, then macro m's dependent tail.
        def emit_head(m):
            # four per-subtile DMAs: transposes of subtile j only wait on
            # load j, so the pipeline starts (and recovers) faster
            x16 = sb.tile([128, 2048], fp16, tag="x16")
            for j in range(4):
                nc.gpsimd.dma_start(
                    out=x16[:, ts(j, 512)], in_=xin[m, :, j]
                )
            return x16

        x16_cur = emit_head(0)
        x16_next = emit_head(1) if n_macro > 1 else None
        prev = None  # state of macro m-1: dict(pg=[...], m=...)

        for m in range(n_macro):
            x16 = x16_cur
            if tmode == "pe":
                # ---- transposes: two features packed per fp32 word; the
                # feature permutation is folded into the host weight layout
                x32 = x16[:].bitcast(fp32)  # [128, 1024]
                xt = sb.tile([128, 1024], fp32, tag="xt")
                for wb in range(2):
                    pxt = ps_xt.tile([128, 512], fp32, tag="pxt")
                    for j in range(4):
                        nc.tensor.transpose(
                            pxt[:, ts(j, 128)],
                            x32[:, j * 256 + wb * 128 : j * 256 + (wb + 1) * 128],
                            ident_sb[:],
                        )
                    nc.vector.tensor_copy(xt[:, ts(wb, 512)], pxt[:])
                # fp16 col = wb*1024 + j*256 + 2*b + h ; K-chunk kk=(wb,h)
                xtr = xt[:].bitcast(fp16).rearrange(
                    "p (wb j b two) -> p wb j b two", wb=2, j=4, two=2
                )
                rhs = [xtr[:, kk // 2, :, :, kk % 2] for kk in range(4)]
            else:
                # ---- transposes on the DMA xbar: frees TensorE entirely
                xt = sb.tile([128, 2048], fp16, tag="xt")
                for k in range(4):
                    for j in range(4):
                        nc.sync.dma_start(
                            out=xt[:, k * 512 + j * 128 : k * 512 + (j + 1) * 128],
                            in_=x16[:, j * 512 + k * 128 : j * 512 + (k + 1) * 128],
                            transpose=True,
                        )
                rhs = [xt[:, ts(k, 512)] for k in range(4)]

            # ---- d1 psum: [128 feat, 512 batch] ----
            pd1 = ps_mm.tile([128, 512], fp32, tag="pmm")
            for k in range(4):
                nc.tensor.matmul(
                    pd1[:],
                    wbig_sb[:, k * 512 : k * 512 + 128],
                    rhs[k],
                    start=(k == 0),
                    stop=(k == 3),
                )

            # ---- previous macro: evict gates (ACT) while PE runs gates_x
            if prev is not None:
                si = sb.tile([128, 512], fp16, tag="si")
                nc.scalar.activation(si[:], prev["pg"][0][:], AF.Sigmoid)
                tg = sb.tile([128, 512], fp16, tag="tg")
                nc.scalar.activation(tg[:], prev["pg"][1][:], AF.Tanh)
                so = sb.tile([128, 512], fp16, tag="so")
                nc.scalar.activation(so[:], prev["pg"][2][:], AF.Sigmoid)
                prev.update(si=si, tg=tg, so=so)

            # ---- gates_x (12 MMs, dep only on xt) ----
            pg = []
            for g in range(3):
                p = ps_g.tile([128, 512], fp32, tag="pg")
                for k in range(4):
                    nc.tensor.matmul(
                        p[:],
                        wbig_sb[:, k * 512 + 128 + g * 128 : k * 512 + 256 + g * 128],
                        rhs[k],
                        start=(k == 0),
                        stop=False,
                        skip_group_check=True,
                    )
                pg.append(p)

            # ---- this macro: d1 relu on ACT; previous macro: lstm cell math
            d1 = sb.tile([128, 512], fp16, tag="d1")
            nc.scalar.activation(d1[:], pd1[:], AF.Relu)
            if prev is not None:
                c = sb.tile([128, 512], fp16, tag="c")
                nc.vector.tensor_mul(c[:], prev["si"][:], prev["tg"][:])
                tc_ = sb.tile([128, 512], fp16, tag="tc")
                nc.scalar.activation(tc_[:], c[:], AF.Tanh)
                h = sb.tile([128, 512], fp16, tag="h")
                nc.vector.tensor_mul(h[:], prev["so"][:], tc_[:])
                prev["h"] = h

            # ---- fc2 ----
            pd2 = ps_mm.tile([64, 512], fp32, tag="pmm")
            nc.tensor.matmul(pd2[:], fc2t_sb[:], d1[:], start=True, stop=True)

            # ---- previous macro: final fc + store ----
            if prev is not None:
                pout = ps_mm.tile([128, 512], fp32, tag="pmm")
                for j in range(4):
                    nc.tensor.matmul(
                        pout[:, ts(j, 128)],
                        prev["h"][:, ts(j, 128)],
                        fct_sb[:],
                        start=True,
                        stop=True,
                    )
                osb = sb.tile([128, 512], fp32, tag="osb")
                nc.vector.tensor_copy(osb[:], pout[:])
                nc.sync.dma_start(out=outr[prev["m"]], in_=osb[:])

            # ---- d2 relu on DVE, then gate d-contributions close groups ----
            d2 = sb.tile([64, 512], fp16, tag="d2")
            nc.vector.tensor_scalar_max(d2[:], pd2[:], 0.0)
            for g in range(3):
                nc.tensor.matmul(
                    pg[g][:],
                    wgdt_sb[:, ts(g, 128)],
                    d2[:],
                    start=False,
                    stop=True,
                    skip_group_check=True,
                )

            prev = {"pg": pg, "m": m}
            x16_cur = x16_next
            x16_next = emit_head(m + 2) if m + 2 < n_macro else None

        # ---- epilogue: finish the last macro ----
        si = sb.tile([128, 512], fp16, tag="si")
        nc.scalar.activation(si[:], prev["pg"][0][:], AF.Sigmoid)
        tg = sb.tile([128, 512], fp16, tag="tg")
        nc.scalar.activation(tg[:], prev["pg"][1][:], AF.Tanh)
        so = sb.tile([128, 512], fp16, tag="so")
        nc.scalar.activation(so[:], prev["pg"][2][:], AF.Sigmoid)
        c = sb.tile([128, 512], fp16, tag="c")
        nc.vector.tensor_mul(c[:], si[:], tg[:])
        tc_ = sb.tile([128, 512], fp16, tag="tc")
        nc.scalar.activation(tc_[:], c[:], AF.Tanh)
        h = sb.tile([128, 512], fp16, tag="h")
        nc.vector.tensor_mul(h[:], so[:], tc_[:])
        pout = ps_mm.tile([128, 512], fp32, tag="pmm")
        for j in range(4):
            nc.tensor.matmul(
                pout[:, ts(j, 128)], h[:, ts(j, 128)], fct_sb[:],
                start=True, stop=True,
            )
        osb = sb.tile([128, 512], fp32, tag="osb")
        nc.vector.tensor_copy(osb[:], pout[:])
        nc.sync.dma_start(out=outr[prev["m"]], in_=osb[:])

    nc.compile()
    return nc


def _prep_consts(fc1_w, fc2_w, fc3_w, W_ih, fc_w, tmode="pe"):
    wi = W_ih[0:128, :IN]
    wg = W_ih[256:384, :IN]
    wo = W_ih[384:512, :IN]
    wbigT = np.concatenate([fc1_w, wi, wg, wo], axis=0).T.astype(np.float32)
    # feature permutation matching the packed-pair transpose: K-chunk
    # kk=(wb,h) row p holds feature 2*(wb*128+p)+h
    if tmode == "pe":
        perm = np.empty(IN, np.int64)
        for kk in range(4):
            wb, h = kk // 2, kk % 2
            p = np.arange(128)
            perm[kk * 128 + p] = 2 * (wb * 128 + p) + h
        wbigT = wbigT[perm]
    # [512, 512] -> [128, 2048] with col k*512+m, row p <-> wbigT[k*128+p, m]
    wbig = np.ascontiguousarray(
        wbigT.reshape(4, 128, 512).transpose(1, 0, 2).reshape(128, 2048)
    ).astype(np.float16)

    wd = np.concatenate(
        [W_ih[0:128, IN:], W_ih[256:384, IN:], W_ih[384:512, IN:]], axis=0
    )  # [384, 128]
    wgdt = np.ascontiguousarray((wd @ fc3_w).T).astype(np.float16)  # [64, 384]
    fc2t = np.ascontiguousarray(fc2_w.T).astype(np.float16)
    fct = np.ascontiguousarray(fc_w.T).astype(np.float16)
    return wbig, fc2t, wgdt, fct


def _numpy_fallback(x, fc1_w, fc1_b, fc2_w, fc2_b, fc3_w, fc3_b,
                    W_ih, b_ih, b_hh, fc_w, fc_b):
    def sig(v):
        return 1.0 / (1.0 + np.exp(-v))

    out = np.empty((x.shape[0], OUT), np.float32)
    bs = 8192
    for s in range(0, x.shape[0], bs):
        xe = x[s : s + bs]
        d = np.maximum(xe @ fc1_w.T + fc1_b, 0.0)
        d = np.maximum(d @ fc2_w.T + fc2_b, 0.0)
        d = d @ fc3_w.T + fc3_b
        xc = np.concatenate([xe, d], axis=1)
        gates = xc @ W_ih.T + b_ih + b_hh
        i, f, g, o = np.split(gates, 4, axis=1)
        c = sig(i) * np.tanh(g)
        h = sig(o) * np.tanh(c)
        out[s : s + bs] = h @ fc_w.T + fc_b
    return out


def kernel(x, fc1_w, fc1_b, fc2_w, fc2_b, fc3_w, fc3_b,
           W_ih, W_hh, b_ih, b_hh, attn_w, attn_b, fc_w, fc_b):
    x = np.asarray(x, np.float32)
    arrs = dict(
        fc1_w=np.asarray(fc1_w, np.float32), fc1_b=np.asarray(fc1_b, np.float32),
        fc2_w=np.asarray(fc2_w, np.float32), fc2_b=np.asarray(fc2_b, np.float32),
        fc3_w=np.asarray(fc3_w, np.float32), fc3_b=np.asarray(fc3_b, np.float32),
        W_ih=np.asarray(W_ih, np.float32), b_ih=np.asarray(b_ih, np.float32),
        b_hh=np.asarray(b_hh, np.float32), fc_w=np.asarray(fc_w, np.float32),
        fc_b=np.asarray(fc_b, np.float32),
    )
    # attn_w/attn_b/W_hh provably never affect the output (softmax over one
    # score == 1; h0 == c0 == 0).  Nonzero biases take the exact host path.
    if any(
        np.any(arrs[k]) for k in ("fc1_b", "fc2_b", "fc3_b", "b_ih", "b_hh", "fc_b")
    ):
        return _numpy_fallback(x, **{k: arrs[k] for k in arrs})

    tmode = "pe"
    wbig, fc2t, wgdt, fct = _prep_consts(
        arrs["fc1_w"], arrs["fc2_w"], arrs["fc3_w"], arrs["W_ih"], arrs["fc_w"],
        tmode,
    )
    ident = np.eye(128, dtype=np.float32)

    n_rows = x.shape[0] // NCORES
    if (n_rows, tmode) not in _nc_cache:
        _nc_cache[(n_rows, tmode)] = _build(n_rows, tmode)
    nc = _nc_cache[(n_rows, tmode)]

    _ensure_path()
    from concourse.bass_utils import run_bass_kernel_spmd

    xs = np.ascontiguousarray(x.reshape(NCORES, n_rows, IN))
    in_maps = [
        {"x": xs[i], "wbig": wbig, "fc2t": fc2t, "wgdt": wgdt, "fct": fct,
         "ident": ident}
        for i in range(NCORES)
    ]
    res = run_bass_kernel_spmd(nc, in_maps, core_ids=list(range(NCORES)))
    return np.concatenate([res.results[i]["out"] for i in range(NCORES)], axis=0)


# revision 24
# speedup vs baseline: 4.2683x; 1.0053x over previous
"""Trainium2 Bass kernel for DNNLSTMWithAttention.

Math (reference collapsed):
  d1 = relu(x @ fc1_w.T + fc1_b)
  d2 = relu(d1 @ fc2_w.T + fc2_b)
  d3 = d2 @ fc3_w.T + fc3_b
  gates = [x, d3] @ W_ih.T + b_ih + b_hh     (PyTorch order i,f,g,o; f unused)
  c = sigmoid(i) * tanh(g)
  h = sigmoid(o) * tanh(c)
  out = h @ fc_w.T + fc_b                    (attention over seq_len=1 == identity)

Structural simplifications (exact for ALL inputs):
  - softmax over a single score == 1 -> ctx == h; attn_w/attn_b never matter
  - f gate multiplies c0 == 0 -> never computed
  - W_hh @ h0 == 0 -> W_hh never matters
When all biases are zero (true for this problem's setup_inputs), fc3 folds
into the gate weights: gates_d = (W_ihd @ fc3_w) @ d2.

Layout: pure data parallel over 8 cores (16384 rows each).  Per core,
512-row macro tiles.  x is DMA-loaded with fp32->fp16 cast (SWDGE), then
transposed on-chip by TensorE (128x128 blocks vs fp16 identity) so features
land on partitions.  All matmuls run in fp16 with fp32 PSUM accumulation;
weights are tiny, pre-transposed on host, and resident in SBUF.
"""

import numpy as np

IN, OUT, H = 512, 128, 128
B = 131072
NCORES = 8
ROWS_PER_CORE = B // NCORES  # 16384
MACRO = 512                  # batch rows per macro tile (4 subtiles of 128)

_nc_cache: dict = {}


def _ensure_path():
    import sys
    try:
        import concourse.bass  # noqa: F401
    except ImportError:
        sys.path.insert(0, "/opt/trn_rl_repo")


def _build(n_rows: int, tmode: str = "pe"):
    """Build the SPMD Bass program for one core processing n_rows rows.

    tmode='pe':  transpose x via TensorE fp32 packed-pair blocks
    tmode='dma': transpose x via DMA xbar (16 fp16 128x128 blocks/macro)
    """
    _ensure_path()
    from contextlib import ExitStack

    import concourse.bass as bass
    import concourse.tile as tile
    from concourse import bacc, mybir

    fp16 = mybir.dt.float16
    fp32 = mybir.dt.float32
    AF = mybir.ActivationFunctionType
    ts = bass.ts

    assert n_rows % MACRO == 0
    n_macro = n_rows // MACRO

    nc = bacc.Bacc(
        "TRN2", target_bir_lowering=False, debug=False, num_devices=NCORES
    )

    x = nc.dram_tensor("x", [n_rows, IN], fp32, kind="ExternalInput").ap()
    # wbig: [128, 4*512]; col k*512+m, row p  <->  W_bigT[k*128+p, m]
    # where W_big rows m = [fc1_w | W_ih_i | W_ih_g | W_ih_o][:, :512]
    wbig = nc.dram_tensor("wbig", [128, 2048], fp16, kind="ExternalInput").ap()
    fc2t = nc.dram_tensor("fc2t", [128, 64], fp16, kind="ExternalInput").ap()
    # (W_ihd[i,g,o] @ fc3_w).T : [64, 384]
    wgdt = nc.dram_tensor("wgdt", [64, 384], fp16, kind="ExternalInput").ap()
    fct = nc.dram_tensor("fct", [H, OUT], fp16, kind="ExternalInput").ap()
    ident = nc.dram_tensor("ident", [128, 128], fp32, kind="ExternalInput").ap()
    out = nc.dram_tensor("out", [n_rows, OUT], fp32, kind="ExternalOutput").ap()

    # per-macro views: partition = row within 128-subtile, free = (subtile j, col)
    xin = x.rearrange("(m j p) f -> m p j f", j=4, p=128)
    outr = out.rearrange("(m j p) o -> m p j o", j=4, p=128)

    with tile.TileContext(nc) as tc, ExitStack() as ctx:
        cpool = ctx.enter_context(tc.tile_pool(name="consts", bufs=1))
        sb = ctx.enter_context(tc.tile_pool(name="sb", bufs=4))
        ps_xt = ctx.enter_context(tc.tile_pool(name="ps_xt", bufs=2, space="PSUM"))
        ps_g = ctx.enter_context(tc.tile_pool(name="ps_g", bufs=3, space="PSUM"))
        ps_mm = ctx.enter_context(tc.tile_pool(name="ps_mm", bufs=3, space="PSUM"))

        # ident first: the very first PE transposes wait only on it + x(0,0)
        ident_sb = cpool.tile([128, 128], fp32, tag="ident")
        nc.sync.dma_start(ident_sb[:], ident)
        wbig_sb = cpool.tile([128, 2048], fp16, tag="wbig")
        nc.sync.dma_start(wbig_sb[:], wbig)
        fc2t_sb = cpool.tile([128, 64], fp16, tag="fc2t")
        nc.sync.dma_start(fc2t_sb[:], fc2t)
        wgdt_sb = cpool.tile([64, 384], fp16, tag="wgdt")
        nc.sync.dma_start(wgdt_sb[:], wgdt)
        fct_sb = cpool.tile([H, OUT], fp16, tag="fct")
        nc.sync.dma_start(fct_sb[:], fct)

        # Software-pipelined emission.  Engine instruction streams execute
        # in order, so emission order is the schedule: each iteration emits
        # macro m's dependency-light head (transposes, d1, gates_x) first,
        # finishes macro m-1 in the gaps, then macro m's dependent tail.
        def emit_head(m):
            # four per-subtile DMAs: transposes of subtile j only wait on
            # load j, so the pipeline starts (and recovers) faster
            x16 = sb.tile([128, 2048], fp16, tag="x16")
            for j in range(4):
                nc.gpsimd.dma_start(
                    out=x16[:, ts(j, 512)], in_=xin[m, :, j]
                )
            return x16

        x16_cur = emit_head(0)
        x16_next = emit_head(1) if n_macro > 1 else None
        prev = None  # state of macro m-1: dict(pg=[...], m=...)

        for m in range(n_macro):
            x16 = x16_cur
            if tmode == "pe":
                # ---- transposes: two features packed per fp32 word; the
                # feature permutation is folded into the host weight layout
                x32 = x16[:].bitcast(fp32)  # [128, 1024]
                xt = sb.tile([128, 1024], fp32, tag="xt")
                for wb in range(2):
                    pxt = ps_xt.tile([128, 512], fp32, tag="pxt")
                    for j in range(4):
                        nc.tensor.transpose(
                            pxt[:, ts(j, 128)],
                            x32[:, j * 256 + wb * 128 : j * 256 + (wb + 1) * 128],
                            ident_sb[:],
                        )
                    if wb == 0:
                        nc.scalar.activation(
                            xt[:, ts(wb, 512)], pxt[:], AF.Identity
                        )
                    else:
                        nc.vector.tensor_copy(xt[:, ts(wb, 512)], pxt[:])
                # fp16 col = wb*1024 + j*256 + 2*b + h ; K-chunk kk=(wb,h)
                xtr = xt[:].bitcast(fp16).rearrange(
                    "p (wb j b two) -> p wb j b two", wb=2, j=4, two=2
                )
                rhs = [xtr[:, kk // 2, :, :, kk % 2] for kk in range(4)]
            else:
                # ---- transposes on the DMA xbar: frees TensorE entirely
                xt = sb.tile([128, 2048], fp16, tag="xt")
                for k in range(4):
                    for j in range(4):
                        nc.sync.dma_start(
                            out=xt[:, k * 512 + j * 128 : k * 512 + (j + 1) * 128],
                            in_=x16[:, j * 512 + k * 128 : j * 512 + (k + 1) * 128],
                            transpose=True,
                        )
                rhs = [xt[:, ts(k, 512)] for k in range(4)]

            # ---- d1 psum: [128 feat, 512 batch] ----
            pd1 = ps_mm.tile([128, 512], fp32, tag="pmm")
            for k in range(4):
                nc.tensor.matmul(
                    pd1[:],
                    wbig_sb[:, k * 512 : k * 512 + 128],
                    rhs[k],
                    start=(k == 0),
                    stop=(k == 3),
                )

            # ---- previous macro: evict gates (ACT) while PE runs gates_x
            if prev is not None:
                si = sb.tile([128, 512], fp16, tag="si")
                nc.scalar.activation(si[:], prev["pg"][0][:], AF.Sigmoid)
                tg = sb.tile([128, 512], fp16, tag="tg")
                nc.scalar.activation(tg[:], prev["pg"][1][:], AF.Tanh)
                so = sb.tile([128, 512], fp16, tag="so")
                nc.scalar.activation(so[:], prev["pg"][2][:], AF.Sigmoid)
                prev.update(si=si, tg=tg, so=so)

            # ---- gates_x (12 MMs, dep only on xt) ----
            pg = []
            for g in range(3):
                p = ps_g.tile([128, 512], fp32, tag="pg")
                for k in range(4):
                    nc.tensor.matmul(
                        p[:],
                        wbig_sb[:, k * 512 + 128 + g * 128 : k * 512 + 256 + g * 128],
                        rhs[k],
                        start=(k == 0),
                        stop=False,
                        skip_group_check=True,
                    )
                pg.append(p)

            # ---- this macro: d1 relu on ACT; previous macro: lstm cell math
            d1 = sb.tile([128, 512], fp16, tag="d1")
            nc.scalar.activation(d1[:], pd1[:], AF.Relu)
            if prev is not None:
                c = sb.tile([128, 512], fp16, tag="c")
                nc.vector.tensor_mul(c[:], prev["si"][:], prev["tg"][:])
                tc_ = sb.tile([128, 512], fp16, tag="tc")
                nc.scalar.activation(tc_[:], c[:], AF.Tanh)
                h = sb.tile([128, 512], fp16, tag="h")
                nc.vector.tensor_mul(h[:], prev["so"][:], tc_[:])
                prev["h"] = h

            # ---- fc2 ----
            pd2 = ps_mm.tile([64, 512], fp32, tag="pmm")
            nc.tensor.matmul(pd2[:], fc2t_sb[:], d1[:], start=True, stop=True)

            # ---- previous macro: final fc + store ----
            if prev is not None:
                pout = ps_mm.tile([128, 512], fp32, tag="pmm")
                for j in range(4):
                    nc.tensor.matmul(
                        pout[:, ts(j, 128)],
                        prev["h"][:, ts(j, 128)],
                        fct_sb[:],
                        start=True,
                        stop=True,
                    )
                osb = sb.tile([128, 512], fp32, tag="osb")
                nc.vector.tensor_copy(osb[:], pout[:])
                nc.sync.dma_start(out=outr[prev["m"]], in_=osb[:])

            # ---- d2 relu on DVE, then gate d-contributions close groups ----
            d2 = sb.tile([64, 512], fp16, tag="d2")
            nc.vector.tensor_scalar_max(d2[:], pd2[:], 0.0)
            for g in range(3):
                nc.tensor.matmul(
                    pg[g][:],
                    wgdt_sb[:, ts(g, 128)],
                    d2[:],
                    start=False,
                    stop=True,
                    skip_group_check=True,
                )

            prev = {"pg": pg, "m": m}
            x16_cur = x16_next
            x16_next = emit_head(m + 2) if m + 2 < n_macro else None

        # ---- epilogue: finish the last macro ----
        si = sb.tile([128, 512], fp16, tag="si")
        nc.scalar.activation(si[:], prev["pg"][0][:], AF.Sigmoid)
        tg = sb.tile([128, 512], fp16, tag="tg")
        nc.scalar.activation(tg[:], prev["pg"][1][:], AF.Tanh)
        so = sb.tile([128, 512], fp16, tag="so")
        nc.scalar.activation(so[:], prev["pg"][2][:], AF.Sigmoid)
        c = sb.tile([128, 512], fp16, tag="c")
        nc.vector.tensor_mul(c[:], si[:], tg[:])
        tc_ = sb.tile([128, 512], fp16, tag="tc")
        nc.scalar.activation(tc_[:], c[:], AF.Tanh)
        h = sb.tile([128, 512], fp16, tag="h")
        nc.vector.tensor_mul(h[:], so[:], tc_[:])
        pout = ps_mm.tile([128, 512], fp32, tag="pmm")
        for j in range(4):
            nc.tensor.matmul(
                pout[:, ts(j, 128)], h[:, ts(j, 128)], fct_sb[:],
                start=True, stop=True,
            )
        osb = sb.tile([128, 512], fp32, tag="osb")
        nc.vector.tensor_copy(osb[:], pout[:])
        nc.sync.dma_start(out=outr[prev["m"]], in_=osb[:])

    nc.compile()
    return nc


def _prep_consts(fc1_w, fc2_w, fc3_w, W_ih, fc_w, tmode="pe"):
    wi = W_ih[0:128, :IN]
    wg = W_ih[256:384, :IN]
    wo = W_ih[384:512, :IN]
    wbigT = np.concatenate([fc1_w, wi, wg, wo], axis=0).T.astype(np.float32)
    # feature permutation matching the packed-pair transpose: K-chunk
    # kk=(wb,h) row p holds feature 2*(wb*128+p)+h
    if tmode == "pe":
        perm = np.empty(IN, np.int64)
        for kk in range(4):
            wb, h = kk // 2, kk % 2
            p = np.arange(128)
            perm[kk * 128 + p] = 2 * (wb * 128 + p) + h
        wbigT = wbigT[perm]
    # [512, 512] -> [128, 2048] with col k*512+m, row p <-> wbigT[k*128+p, m]
    wbig = np.ascontiguousarray(
        wbigT.reshape(4, 128, 512).transpose(1, 0, 2).reshape(128, 2048)
    ).astype(np.float16)

    wd = np.concatenate(
        [W_ih[0:128, IN:], W_ih[256:384, IN:], W_ih[384:512, IN:]], axis=0
    )  # [384, 128]
    wgdt = np.ascontiguousarray((wd @ fc3_w).T).astype(np.float16)  # [64, 384]
    fc2t = np.ascontiguousarray(fc2_w.T).astype(np.float16)
    fct = np.ascontiguousarray(fc_w.T).astype(np.float16)
    return wbig, fc2t, wgdt, fct


def _numpy_fallback(x, fc1_w, fc1_b, fc2_w, fc2_b, fc3_w, fc3_b,
                    W_ih, b_ih, b_hh, fc_w, fc_b):
    def sig(v):
        return 1.0 / (1.0 + np.exp(-v))

    out = np.empty((x.shape[0], OUT), np.float32)
    bs = 8192
    for s in range(0, x.shape[0], bs):
        xe = x[s : s + bs]
        d = np.maximum(xe @ fc1_w.T + fc1_b, 0.0)
        d = np.maximum(d @ fc2_w.T + fc2_b, 0.0)
        d = d @ fc3_w.T + fc3_b
        xc = np.concatenate([xe, d], axis=1)
        gates = xc @ W_ih.T + b_ih + b_hh
        i, f, g, o = np.split(gates, 4, axis=1)
        c = sig(i) * np.tanh(g)
        h = sig(o) * np.tanh(c)
        out[s : s + bs] = h @ fc_w.T + fc_b
    return out


def kernel(x, fc1_w, fc1_b, fc2_w, fc2_b, fc3_w, fc3_b,
           W_ih, W_hh, b_ih, b_hh, attn_w, attn_b, fc_w, fc_b):
    x = np.asarray(x, np.float32)
    arrs = dict(
        fc1_w=np.asarray(fc1_w, np.float32), fc1_b=np.asarray(fc1_b, np.float32),
        fc2_w=np.asarray(fc2_w, np.float32), fc2_b=np.asarray(fc2_b, np.float32),
        fc3_w=np.asarray(fc3_w, np.float32), fc3_b=np.asarray(fc3_b, np.float32),
        W_ih=np.asarray(W_ih, np.float32), b_ih=np.asarray(b_ih, np.float32),
        b_hh=np.asarray(b_hh, np.float32), fc_w=np.asarray(fc_w, np.float32),
        fc_b=np.asarray(fc_b, np.float32),
    )
    # attn_w/attn_b/W_hh provably never affect the output (softmax over one
    # score == 1; h0 == c0 == 0).  Nonzero biases take the exact host path.
    if any(
        np.any(arrs[k]) for k in ("fc1_b", "fc2_b", "fc3_b", "b_ih", "b_hh", "fc_b")
    ):
        return _numpy_fallback(x, **{k: arrs[k] for k in arrs})

    tmode = "pe"
    wbig, fc2t, wgdt, fct = _prep_consts(
        arrs["fc1_w"], arrs["fc2_w"], arrs["fc3_w"], arrs["W_ih"], arrs["fc_w"],
        tmode,
    )
    ident = np.eye(128, dtype=np.float32)

    n_rows = x.shape[0] // NCORES
    if (n_rows, tmode) not in _nc_cache:
        _nc_cache[(n_rows, tmode)] = _build(n_rows, tmode)
    nc = _nc_cache[(n_rows, tmode)]

    _ensure_path()
    from concourse.bass_utils import run_bass_kernel_spmd

    xs = np.ascontiguousarray(x.reshape(NCORES, n_rows, IN))
    in_maps = [
        {"x": xs[i], "wbig": wbig, "fc2t": fc2t, "wgdt": wgdt, "fct": fct,
         "ident": ident}
        for i in range(NCORES)
    ]
    res = run_bass_kernel_spmd(nc, in_maps, core_ids=list(range(NCORES)))
    return np.concatenate([res.results[i]["out"] for i in range(NCORES)], axis=0)
